# revision 20
# baseline (speedup 1.0000x reference)
"""Banded (sliding-window) multi-head attention for Trainium2, 8 NeuronCores.

Problem: x[4, 2048, 512] -> QKV proj -> RoPE -> banded attention
(window [q-127, q+128]) -> out proj.  See reference.py.

Sharding: (batch n, head-half) -> 8 cores.  Each core computes 4 heads of one
batch end-to-end and a partial out-projection (contraction over its 256 hidden
dims); host gather sums the two partials per batch and adds the bias.

On-core pipeline (matmuls bf16, fp32 PSUM accumulation for qk):
  qkT  = Wqk^T.T @ xT        (feat-major, 2 head-packs of 128 partitions)
  RoPE via signed-permutation matmul (rotT = R.T.T @ qkT) + DVE/gpsimd mul/add
  v    = xT.T @ WvT          (token-major, 16 key chunks, ones col for sums)
  scores, key-chunk-stationary: per key chunk rc, scoresT[k, 3 q-tiles]
         = kT_rc.T @ qT[rc-1..rc+1]  (two row-tiled head matmuls, N<=384)
  expT = exp(scoresT) both heads in one activation; band masks applied
         multiplicatively on DVE (bf16 constant tiles)
  attn[q, d] (+ sums via ones column in v) = expT.T @ v   per q tile
  normalize: recip(sums) -> per-partition scale (DVE a=0, ACT a=1)
  PE-transpose attn[q, (2 heads x 64d)] -> attnT[d-pack, q]
  out partial = attnT.T @ owT -> bf16 psum -> DMA out bf16, host upcasts
"""

import numpy as np
import ml_dtypes

import concourse.bass as bass
import concourse.bacc as bacc
import concourse.mybir as mybir
import concourse.tile as tile
from concourse import bass_utils

# ---------------- problem constants (hardcoded per contract) ----------------
N_BATCH = 4
T = 2048
D_MODEL = 512
NHEAD = 8
HEAD_DIM = 64           # also rotary dim
WIN_LO, WIN_HI = 127, 128
N_CORES = 8

NT = T // 128           # 16 query tiles / key chunks of 128
BF = mybir.dt.bfloat16
F8 = mybir.dt.float8e4
F32 = mybir.dt.float32

_CACHE = {}


# ---------------- host-side constant prep ----------------
def _bf16(a):
    return np.ascontiguousarray(a, dtype=np.float32).astype(ml_dtypes.bfloat16)


def _fp8(a):
    return np.ascontiguousarray(a, dtype=np.float32).astype(ml_dtypes.float8_e4m3fn)


def _rope_tables():
    # row p of a 128-partition head-pack corresponds to head dim d = p % 64
    d_idx = np.arange(128) % HEAD_DIM
    f_idx = d_idx % (HEAD_DIM // 2)
    invf = 1.0 / (10000.0 ** (np.arange(0, HEAD_DIM, 2, dtype=np.float32) / HEAD_DIM))
    ang = np.arange(T, dtype=np.float32)[None, :] * invf[f_idx][:, None]  # [128, T]
    return _bf16(np.cos(ang)), _bf16(np.sin(ang))


def _rot_matrix_T():
    # rot_qT = R @ qT with R the rotate_half signed permutation (per 64-dim head)
    R = np.zeros((128, 128), np.float32)
    for p in range(128):
        if p % 64 < 32:
            R[p, p + 32] = -1.0
        else:
            R[p, p - 32] = 1.0
    return _bf16(R.T)


def _mask_blocks():
    # expT_rc block b holds scoresT[key chunk rc, q tile rc-1+b]; k - q offset
    # is 128*(1-b).  Block 0 (keys one chunk right of queries): keep kp <= qp.
    # Block 2 (keys one chunk left): keep kp >= qp+1.  Block 1 (diag): in-band.
    kp = np.arange(128)[:, None]
    qp = np.arange(128)[None, :]
    m0 = (kp <= qp).astype(np.float32)        # right chunk
    m1 = np.ones((128, 128), np.float32)      # diagonal, fully in band
    m2 = (kp >= qp + 1).astype(np.float32)    # left chunk
    m = np.stack([m0, m1, m2])                # [3, 128, 128]
    # duplicate for both heads of a pack: [128, 2(a), 3(b), 128]
    return _bf16(np.broadcast_to(m[None], (2, 3, 128, 128)).transpose(2, 0, 1, 3))


def _prep_weights(Wqkv_w, out_w, half):
    hs = [half * 4 + i for i in range(4)]
    Wq = Wqkv_w[0 * D_MODEL:1 * D_MODEL].reshape(NHEAD, HEAD_DIM, D_MODEL)[hs]
    Wk = Wqkv_w[1 * D_MODEL:2 * D_MODEL].reshape(NHEAD, HEAD_DIM, D_MODEL)[hs]
    Wv = Wqkv_w[2 * D_MODEL:3 * D_MODEL].reshape(NHEAD, HEAD_DIM, D_MODEL)[hs]
    Wq = Wq * np.float32(1.0 / np.sqrt(HEAD_DIM))     # fold attention scale
    # feat order: q(h0,h1), q(h2,h3), k(h0,h1), k(h2,h3)
    wqk = np.concatenate([Wq.reshape(2, 128, D_MODEL), Wk.reshape(2, 128, D_MODEL)], 0)
    wqkT = wqk.reshape(512, D_MODEL).T.reshape(4, 128, 512)       # [xfeat c, 128, feat]
    wvT = Wv.reshape(256, D_MODEL).T.reshape(4, 128, 256)
    owT = out_w[:, half * 256:(half + 1) * 256].T.reshape(2, 128, 512)
    return _bf16(wqkT), _bf16(wvT), _bf16(owT)


# ---------------- bass program ----------------
def build_nc():
    nc = bacc.Bacc("TRN2", debug=False, enable_asserts=False,
                   target_bir_lowering=False)

    xT_d = nc.dram_tensor("xT", [4, 128, T], BF, kind="ExternalInput")
    wqk_d = nc.dram_tensor("wqk", [4, 128, 512], BF, kind="ExternalInput")
    wv_d = nc.dram_tensor("wv", [4, 128, 256], BF, kind="ExternalInput")
    ow_d = nc.dram_tensor("ow", [2, 128, 512], BF, kind="ExternalInput")
    cos_d = nc.dram_tensor("cosT", [128, T], BF, kind="ExternalInput")
    sin_d = nc.dram_tensor("sinT", [128, T], BF, kind="ExternalInput")
    rt_d = nc.dram_tensor("rotT", [128, 128], BF, kind="ExternalInput")
    id_d = nc.dram_tensor("ident", [128, 128], BF, kind="ExternalInput")
    msk_d = nc.dram_tensor("maskblk", [128, 2, 3, 128], BF, kind="ExternalInput")
    out_d = nc.dram_tensor("out", [T, 512], BF, kind="ExternalOutput")

    with tile.TileContext(nc) as tc:
        with (
            tc.tile_pool(name="persist", bufs=1) as pers,
            tc.tile_pool(name="work", bufs=3) as work,
            tc.tile_pool(name="expp", bufs=8) as expp,
            tc.tile_pool(name="psbig", bufs=2, space="PSUM") as psbig,
            tc.tile_pool(name="pssc", bufs=1, space="PSUM") as pssc,
            tc.tile_pool(name="psa", bufs=2, space="PSUM") as psa,
            tc.tile_pool(name="pst", bufs=1, space="PSUM") as pst,
            tc.tile_pool(name="pso", bufs=1, space="PSUM") as pso,
        ):
            # ------- persistent SBUF tensors -------
            xT = pers.tile([128, 4, T], BF)
            wqk = pers.tile([128, 4, 512], BF)
            wv = pers.tile([128, 4, 256], BF)
            ow = pers.tile([128, 2, 512], BF)
            cosb = pers.tile([128, T], BF)
            sinb = pers.tile([128, T], BF)
            rt = pers.tile([128, 128], BF)
            ident = pers.tile([128, 128], BF)
            mskb = pers.tile([128, 2, 3, 128], BF)
            v4 = pers.tile([128, 4, NT, 65], BF)
            qT = pers.tile([128, 2, T], BF)
            kT = pers.tile([128, 2, T], BF)

            # DMA order = first-use order
            # wv descriptor-gen on the scalar queue, in parallel with x0's
            # on sync -- halves the serial DGE latency ahead of the first matmul
            for c in range(4):
                nc.scalar.dma_start(out=wv[:, c, :], in_=wv_d[c])
                nc.sync.dma_start(out=xT[:, c, 0:512], in_=xT_d[c][:, 0:512])
            for c in range(4):
                nc.sync.dma_start(out=wqk[:, c, :], in_=wqk_d[c])
            nc.sync.dma_start(out=rt[:], in_=rt_d[:])
            nc.sync.dma_start(out=cosb[:, 0:512], in_=cos_d[:, 0:512])
            nc.sync.dma_start(out=sinb[:, 0:512], in_=sin_d[:, 0:512])
            for nn in range(1, 4):
                for c in range(4):
                    nc.sync.dma_start(
                        out=xT[:, c, nn * 512:(nn + 1) * 512],
                        in_=xT_d[c][:, nn * 512:(nn + 1) * 512])
            nc.sync.dma_start(out=cosb[:, 512:], in_=cos_d[:, 512:])
            nc.sync.dma_start(out=sinb[:, 512:], in_=sin_d[:, 512:])
            nc.sync.dma_start(out=mskb[:], in_=msk_d[:])
            for c in range(2):
                nc.sync.dma_start(out=ow[:, c, :], in_=ow_d[c])
            nc.sync.dma_start(out=ident[:], in_=id_d[:])

            nc.vector.memset(v4[:, :, :, 64:65], 1.0)   # ones col -> sums

            # ------- production: qkv proj + rope for one 512-token block -----
            def v_tile(t):
                ps_v = psbig.tile([128, 256], F32, tag="big", name="ps_v")
                for c in range(4):
                    nc.tensor.matmul(
                        ps_v[:],
                        lhsT=xT[:, c, t * 128:(t + 1) * 128],
                        rhs=wv[:, c, :],
                        start=(c == 0), stop=(c == 3),
                    )
                nc.vector.tensor_copy(
                    v4[:, :, t, 0:64],
                    ps_v[:].rearrange("p (h d) -> p h d", h=4),
                )

            def qk_block(n):
                # rope for pack m trails the qk matmuls of pack m+1, so the
                # rot matmul never stalls PE on the scalar psum->sbuf cast
                nsl = slice(n * 512, (n + 1) * 512)
                raws = {}

                def rope_tail(m):
                    raw = raws.pop(m)
                    ps_rot = psbig.tile([128, 512], F32, tag="big")
                    nc.tensor.matmul(ps_rot[:], lhsT=rt[:], rhs=raw[:],
                                     start=True, stop=True)
                    t1 = work.tile([128, 512], BF, tag="t1")
                    nc.gpsimd.tensor_mul(t1[:], raw[:], cosb[:, nsl])
                    t2 = work.tile([128, 512], BF, tag="t2")
                    nc.vector.tensor_mul(t2[:], ps_rot[:], sinb[:, nsl])
                    dest = qT[:, m, nsl] if m < 2 else kT[:, m - 2, nsl]
                    if m % 2 == 0:
                        nc.gpsimd.tensor_add(dest, t1[:], t2[:])
                    else:
                        nc.vector.tensor_add(dest, t1[:], t2[:])

                ms = (0, 2, 1, 3)   # hp0's q and k packs first
                for i, m in enumerate(ms):
                    ps_qk = psbig.tile([128, 512], F32, tag="big")
                    for c in range(4):
                        nc.tensor.matmul(
                            ps_qk[:],
                            lhsT=wqk[:, c, m * 128:(m + 1) * 128],
                            rhs=xT[:, c, nsl],
                            start=(c == 0), stop=(c == 3),
                        )
                    raw = work.tile([128, 512], BF, tag="raw")
                    nc.scalar.copy(raw[:], ps_qk[:])
                    raws[m] = raw
                    if i > 0:
                        rope_tail(ms[i - 1])
                rope_tail(ms[3])

            # ------- attention: key-chunk-stationary scores + exp + mask -----
            expT = {}

            def sc_mm(rc, hp):
                lo_t = max(rc - 1, 0)
                hi_t = min(rc + 1, NT - 1)
                b0 = lo_t - (rc - 1)           # 0 or 1
                nb = hi_t - lo_t + 1           # 2 or 3
                csl = slice(b0 * 128, (b0 + nb) * 128)
                ksl = slice(rc * 128, (rc + 1) * 128)
                qsl = slice(lo_t * 128, (hi_t + 1) * 128)
                ps_s = pssc.tile([128, 2, 512], F32, tag="sc")
                for a in range(2):
                    rsl = slice(a * 64, (a + 1) * 64)
                    nc.tensor.matmul(
                        ps_s[:, a, csl],
                        lhsT=kT[rsl, hp, ksl],
                        rhs=qT[rsl, hp, qsl],
                        start=True, stop=True,
                    )
                return ps_s, csl, b0, nb

            def sc_post(rc, hp, ps_s, csl, b0, nb):
                ex = expp.tile([128, 2, 384], BF, tag="expT", name=f"ex{rc}_{hp}")
                nc.scalar.activation(
                    ex[:, :, csl], ps_s[:, :, csl],
                    mybir.ActivationFunctionType.Exp)
                # band masks, multiplicative (both heads in one op);
                # block 1 (diagonal) is fully in-band, no mask needed
                exv = ex[:].rearrange("p a (b q) -> p a b q", q=128)
                if rc > 0 and rc < NT - 1:
                    nc.vector.tensor_mul(
                        exv[:, :, 0:3:2, :], exv[:, :, 0:3:2, :],
                        mskb[:, :, 0:3:2, :])
                elif rc == 0:
                    nc.vector.tensor_mul(
                        exv[:, :, 2:3, :], exv[:, :, 2:3, :],
                        mskb[:, :, 2:3, :])
                else:
                    nc.vector.tensor_mul(
                        exv[:, :, 0:1, :], exv[:, :, 0:1, :],
                        mskb[:, :, 0:1, :])
                expT[(rc, hp)] = ex

            # attention for tile t lags the score pipeline by 2 chunks, so
            # every expT it touches is complete -- its matmuls fill the PE
            # while the current chunk's exp/mask drain on ACT/DVE.
            at_state = {}

            def at_hp(t, hp):
                cs = [c for c in range(3)
                      if not (t == 0 and c == 0) and not (t == NT - 1 and c == 2)]
                if hp == 0:
                    at_state[t] = (
                        work.tile([128, 2, 2, 64], BF, tag="aq", name=f"aq{t}"),
                        pst.tile([128, 2, 128], BF, tag="small", name=f"pst{t}"),
                    )
                aq, ps_t = at_state[t]
                ps_a = psa.tile([128, 2, 65], F32, tag="small2")
                for a in range(2):
                    for i, c in enumerate(cs):
                        ex = expT[(t - 1 + c, hp)]
                        nc.tensor.matmul(
                            ps_a[:, a, :],
                            lhsT=ex[:, a, (2 - c) * 128:(3 - c) * 128],
                            rhs=v4[:, hp * 2 + a, t - 1 + c, :],
                            start=(i == 0), stop=(i == len(cs) - 1),
                        )
                rcp = work.tile([128, 2, 1], F32, tag="rcp")
                nc.vector.reciprocal_approx_fast(rcp[:], ps_a[:, :, 64:65])
                nc.vector.tensor_scalar_mul(
                    aq[:, hp, 0, :], ps_a[:, 0, 0:64], rcp[:, 0, :])
                nc.scalar.mul(aq[:, hp, 1, :], ps_a[:, 1, 0:64], rcp[:, 1, :])

            def at_tr(t, hp):
                aq, ps_t = at_state[t]
                nc.tensor.transpose(ps_t[:, hp, :], aq[:, hp, :, :], ident[:])

            def at_fin(t):
                _, ps_t = at_state.pop(t)
                att = work.tile([128, 2, 128], BF, tag="att")
                nc.vector.tensor_copy(att[:], ps_t[:])
                ps_o = pso.tile([128, 512], F32, tag="out")
                for hp in range(2):
                    nc.tensor.matmul(
                        ps_o[:],
                        lhsT=att[:, hp, :],
                        rhs=ow[:, hp, :],
                        start=(hp == 0), stop=(hp == 1),
                    )
                osb = work.tile([128, 512], BF, tag="osb")
                if t % 2 == 0:
                    nc.vector.tensor_copy(osb[:], ps_o[:])
                else:
                    nc.scalar.copy(osb[:], ps_o[:])
                # tail tiles issue from gpsimd (idle once production ends),
                # dodging the sync queue's serialized descriptor generation
                eng = nc.gpsimd if t >= 12 else nc.sync
                eng.dma_start(out=out_d[t * 128:(t + 1) * 128, :], in_=osb[:])

            def process_rc(rc):
                args0 = sc_mm(rc, 0)
                sc_post(rc, 0, *args0)
                if rc >= 2:
                    at_hp(rc - 2, 0)
                args1 = sc_mm(rc, 1)
                sc_post(rc, 1, *args1)
                if rc >= 2:
                    at_hp(rc - 2, 1)
                    at_tr(rc - 2, 0)
                    at_tr(rc - 2, 1)
                    at_fin(rc - 2)

            # ------- schedule: production block n, then trailing attention ---
            sc_done = -1
            for n in range(4):
                for t in range(4 * n, 4 * n + 4):
                    v_tile(t)
                qk_block(n)
                hi_rc = 4 * n + 2 if n < 3 else NT - 1
                while sc_done < hi_rc:
                    sc_done += 1
                    process_rc(sc_done)
            for t in (NT - 2, NT - 1):
                at_hp(t, 0)
                at_hp(t, 1)
                at_tr(t, 0)
                at_tr(t, 1)
                at_fin(t)

    nc.compile()
    return nc


# ---------------- host prep + run + gather ----------------
def _get_state():
    if "nc" not in _CACHE:
        _CACHE["nc"] = build_nc()
    if "cos" not in _CACHE:
        _CACHE["cos"], _CACHE["sin"] = _rope_tables()
        _CACHE["rotT"] = _rot_matrix_T()
        _CACHE["ident"] = _bf16(np.eye(128, dtype=np.float32))
        _CACHE["maskblk"] = _mask_blocks()
    return _CACHE


def make_in_maps(x, Wqkv_w, out_w):
    st = _get_state()
    halves = [_prep_weights(Wqkv_w, out_w, h) for h in range(2)]
    in_maps = []
    for core in range(N_CORES):
        n, half = core // 2, core % 2
        wqkT, wvT, owT = halves[half]
        xT = _bf16(x[n].T).reshape(4, 128, T)
        in_maps.append({
            "xT": xT, "wqk": wqkT, "wv": wvT, "ow": owT,
            "cosT": st["cos"], "sinT": st["sin"],
            "rotT": st["rotT"], "ident": st["ident"],
            "maskblk": st["maskblk"],
        })
    return in_maps


def gather(results, out_b, dtype):
    outs = []
    for n in range(N_BATCH):
        o = (results[2 * n]["out"].astype(np.float32)
             + results[2 * n + 1]["out"].astype(np.float32) + out_b[None, :])
        outs.append(o)
    return np.stack(outs).astype(dtype, copy=False)


def kernel(x, Wqkv_w, out_w, out_b):
    x = np.asarray(x)
    st = _get_state()
    in_maps = make_in_maps(x, np.asarray(Wqkv_w), np.asarray(out_w))
    res = bass_utils.run_bass_kernel_spmd(
        st["nc"], in_maps, core_ids=list(range(N_CORES)))
    return gather(res.results, np.asarray(out_b), x.dtype)


# revision 21
# speedup vs baseline: 1.0053x; 1.0053x over previous
"""Banded (sliding-window) multi-head attention for Trainium2, 8 NeuronCores.

Problem: x[4, 2048, 512] -> QKV proj -> RoPE -> banded attention
(window [q-127, q+128]) -> out proj.  See reference.py.

Sharding: (batch n, head-half) -> 8 cores.  Each core computes 4 heads of one
batch end-to-end and a partial out-projection (contraction over its 256 hidden
dims); host gather sums the two partials per batch and adds the bias.

On-core pipeline (matmuls bf16, fp32 PSUM accumulation for qk):
  qkT  = Wqk^T.T @ xT        (feat-major, 2 head-packs of 128 partitions)
  RoPE via signed-permutation matmul (rotT = R.T.T @ qkT) + DVE/gpsimd mul/add
  v    = xT.T @ WvT          (token-major, 16 key chunks, ones col for sums)
  scores, key-chunk-stationary: per key chunk rc, scoresT[k, 3 q-tiles]
         = kT_rc.T @ qT[rc-1..rc+1]  (two row-tiled head matmuls, N<=384)
  expT = exp(scoresT) both heads in one activation; band masks applied
         multiplicatively on DVE (bf16 constant tiles)
  attn[q, d] (+ sums via ones column in v) = expT.T @ v   per q tile
  normalize: recip(sums) -> per-partition scale (DVE a=0, ACT a=1)
  PE-transpose attn[q, (2 heads x 64d)] -> attnT[d-pack, q]
  out partial = attnT.T @ owT -> bf16 psum -> DMA out bf16, host upcasts
"""

import numpy as np
import ml_dtypes

import concourse.bass as bass
import concourse.bacc as bacc
import concourse.mybir as mybir
import concourse.tile as tile
from concourse import bass_utils

# ---------------- problem constants (hardcoded per contract) ----------------
N_BATCH = 4
T = 2048
D_MODEL = 512
NHEAD = 8
HEAD_DIM = 64           # also rotary dim
WIN_LO, WIN_HI = 127, 128
N_CORES = 8

NT = T // 128           # 16 query tiles / key chunks of 128
BF = mybir.dt.bfloat16
F8 = mybir.dt.float8e4
F32 = mybir.dt.float32

_CACHE = {}


# ---------------- host-side constant prep ----------------
def _bf16(a):
    return np.ascontiguousarray(a, dtype=np.float32).astype(ml_dtypes.bfloat16)


def _fp8(a):
    return np.ascontiguousarray(a, dtype=np.float32).astype(ml_dtypes.float8_e4m3fn)


def _rope_tables():
    # row p of a 128-partition head-pack corresponds to head dim d = p % 64
    d_idx = np.arange(128) % HEAD_DIM
    f_idx = d_idx % (HEAD_DIM // 2)
    invf = 1.0 / (10000.0 ** (np.arange(0, HEAD_DIM, 2, dtype=np.float32) / HEAD_DIM))
    ang = np.arange(T, dtype=np.float32)[None, :] * invf[f_idx][:, None]  # [128, T]
    return _bf16(np.cos(ang)), _bf16(np.sin(ang))


def _rot_matrix_T():
    # rot_qT = R @ qT with R the rotate_half signed permutation (per 64-dim head)
    R = np.zeros((128, 128), np.float32)
    for p in range(128):
        if p % 64 < 32:
            R[p, p + 32] = -1.0
        else:
            R[p, p - 32] = 1.0
    return _bf16(R.T)


def _mask_blocks():
    # expT_rc block b holds scoresT[key chunk rc, q tile rc-1+b]; k - q offset
    # is 128*(1-b).  Block 0 (keys one chunk right of queries): keep kp <= qp.
    # Block 2 (keys one chunk left): keep kp >= qp+1.  Block 1 (diag): in-band.
    kp = np.arange(128)[:, None]
    qp = np.arange(128)[None, :]
    m0 = (kp <= qp).astype(np.float32)        # right chunk
    m1 = np.ones((128, 128), np.float32)      # diagonal, fully in band
    m2 = (kp >= qp + 1).astype(np.float32)    # left chunk
    m = np.stack([m0, m1, m2])                # [3, 128, 128]
    # duplicate for both heads of a pack: [128, 2(a), 3(b), 128]
    return _bf16(np.broadcast_to(m[None], (2, 3, 128, 128)).transpose(2, 0, 1, 3))


def _prep_weights(Wqkv_w, out_w, half):
    hs = [half * 4 + i for i in range(4)]
    Wq = Wqkv_w[0 * D_MODEL:1 * D_MODEL].reshape(NHEAD, HEAD_DIM, D_MODEL)[hs]
    Wk = Wqkv_w[1 * D_MODEL:2 * D_MODEL].reshape(NHEAD, HEAD_DIM, D_MODEL)[hs]
    Wv = Wqkv_w[2 * D_MODEL:3 * D_MODEL].reshape(NHEAD, HEAD_DIM, D_MODEL)[hs]
    Wq = Wq * np.float32(1.0 / np.sqrt(HEAD_DIM))     # fold attention scale
    # feat order: q(h0,h1), q(h2,h3), k(h0,h1), k(h2,h3)
    wqk = np.concatenate([Wq.reshape(2, 128, D_MODEL), Wk.reshape(2, 128, D_MODEL)], 0)
    wqkT = wqk.reshape(512, D_MODEL).T.reshape(4, 128, 512)       # [xfeat c, 128, feat]
    wvT = Wv.reshape(256, D_MODEL).T.reshape(4, 128, 256)
    owT = out_w[:, half * 256:(half + 1) * 256].T.reshape(2, 128, 512)
    return _bf16(wqkT), _bf16(wvT), _bf16(owT)


# ---------------- bass program ----------------
def build_nc():
    nc = bacc.Bacc("TRN2", debug=False, enable_asserts=False,
                   target_bir_lowering=False)

    xT_d = nc.dram_tensor("xT", [4, 128, T], BF, kind="ExternalInput")
    wqk_d = nc.dram_tensor("wqk", [4, 128, 512], BF, kind="ExternalInput")
    wv_d = nc.dram_tensor("wv", [4, 128, 256], BF, kind="ExternalInput")
    ow_d = nc.dram_tensor("ow", [2, 128, 512], BF, kind="ExternalInput")
    cos_d = nc.dram_tensor("cosT", [128, T], BF, kind="ExternalInput")
    sin_d = nc.dram_tensor("sinT", [128, T], BF, kind="ExternalInput")
    rt_d = nc.dram_tensor("rotT", [128, 128], BF, kind="ExternalInput")
    id_d = nc.dram_tensor("ident", [128, 128], BF, kind="ExternalInput")
    msk_d = nc.dram_tensor("maskblk", [128, 2, 3, 128], BF, kind="ExternalInput")
    out_d = nc.dram_tensor("out", [T, 512], BF, kind="ExternalOutput")

    with tile.TileContext(nc) as tc:
        with (
            tc.tile_pool(name="persist", bufs=1) as pers,
            tc.tile_pool(name="work", bufs=3) as work,
            tc.tile_pool(name="expp", bufs=8) as expp,
            tc.tile_pool(name="psbig", bufs=2, space="PSUM") as psbig,
            tc.tile_pool(name="pssc", bufs=1, space="PSUM") as pssc,
            tc.tile_pool(name="psa", bufs=2, space="PSUM") as psa,
            tc.tile_pool(name="pst", bufs=1, space="PSUM") as pst,
            tc.tile_pool(name="pso", bufs=1, space="PSUM") as pso,
        ):
            # ------- persistent SBUF tensors -------
            xT = pers.tile([128, 4, T], BF)
            wqk = pers.tile([128, 4, 512], BF)
            wv = pers.tile([128, 4, 256], BF)
            ow = pers.tile([128, 2, 512], BF)
            cosb = pers.tile([128, T], BF)
            sinb = pers.tile([128, T], BF)
            rt = pers.tile([128, 128], BF)
            ident = pers.tile([128, 128], BF)
            mskb = pers.tile([128, 2, 3, 128], BF)
            v4 = pers.tile([128, 4, NT, 65], BF)
            qT = pers.tile([128, 2, T], BF)
            kT = pers.tile([128, 2, T], BF)

            # DMA order = first-use order
            # wv descriptor-gen on the scalar queue, in parallel with x0's
            # on sync -- halves the serial DGE latency ahead of the first matmul
            for c in range(4):
                nc.scalar.dma_start(out=wv[:, c, :], in_=wv_d[c])
                nc.sync.dma_start(out=xT[:, c, 0:512], in_=xT_d[c][:, 0:512])
            for c in range(4):
                nc.sync.dma_start(out=wqk[:, c, :], in_=wqk_d[c])
            nc.sync.dma_start(out=rt[:], in_=rt_d[:])
            nc.sync.dma_start(out=cosb[:, 0:512], in_=cos_d[:, 0:512])
            nc.sync.dma_start(out=sinb[:, 0:512], in_=sin_d[:, 0:512])
            for nn in range(1, 4):
                for c in range(4):
                    nc.sync.dma_start(
                        out=xT[:, c, nn * 512:(nn + 1) * 512],
                        in_=xT_d[c][:, nn * 512:(nn + 1) * 512])
            nc.sync.dma_start(out=cosb[:, 512:], in_=cos_d[:, 512:])
            nc.sync.dma_start(out=sinb[:, 512:], in_=sin_d[:, 512:])
            nc.sync.dma_start(out=mskb[:], in_=msk_d[:])
            for c in range(2):
                nc.sync.dma_start(out=ow[:, c, :], in_=ow_d[c])
            nc.sync.dma_start(out=ident[:], in_=id_d[:])

            nc.vector.memset(v4[:, :, :, 64:65], 1.0)   # ones col -> sums

            # ------- production: qkv proj + rope for one 512-token block -----
            def v_tile(t):
                ps_v = psbig.tile([128, 256], F32, tag="big", name="ps_v")
                for c in range(4):
                    nc.tensor.matmul(
                        ps_v[:],
                        lhsT=xT[:, c, t * 128:(t + 1) * 128],
                        rhs=wv[:, c, :],
                        start=(c == 0), stop=(c == 3),
                    )
                nc.vector.tensor_copy(
                    v4[:, :, t, 0:64],
                    ps_v[:].rearrange("p (h d) -> p h d", h=4),
                )

            def qk_block(n):
                # rope for pack m trails the qk matmuls of pack m+1, so the
                # rot matmul never stalls PE on the scalar psum->sbuf cast
                nsl = slice(n * 512, (n + 1) * 512)
                raws = {}

                def rope_tail(m):
                    raw = raws.pop(m)
                    ps_rot = psbig.tile([128, 512], F32, tag="big")
                    nc.tensor.matmul(ps_rot[:], lhsT=rt[:], rhs=raw[:],
                                     start=True, stop=True)
                    t1 = work.tile([128, 512], BF, tag="t1")
                    nc.gpsimd.tensor_mul(t1[:], raw[:], cosb[:, nsl])
                    t2 = work.tile([128, 512], BF, tag="t2")
                    nc.vector.tensor_mul(t2[:], ps_rot[:], sinb[:, nsl])
                    dest = qT[:, m, nsl] if m < 2 else kT[:, m - 2, nsl]
                    if m % 2 == 0:
                        nc.gpsimd.tensor_add(dest, t1[:], t2[:])
                    else:
                        nc.vector.tensor_add(dest, t1[:], t2[:])

                ms = (0, 2, 1, 3)   # hp0's q and k packs first
                for i, m in enumerate(ms):
                    ps_qk = psbig.tile([128, 512], F32, tag="big")
                    for c in range(4):
                        nc.tensor.matmul(
                            ps_qk[:],
                            lhsT=wqk[:, c, m * 128:(m + 1) * 128],
                            rhs=xT[:, c, nsl],
                            start=(c == 0), stop=(c == 3),
                        )
                    raw = work.tile([128, 512], BF, tag="raw")
                    nc.scalar.copy(raw[:], ps_qk[:])
                    raws[m] = raw
                    if i > 0:
                        rope_tail(ms[i - 1])
                rope_tail(ms[3])

            # ------- attention: key-chunk-stationary scores + exp + mask -----
            expT = {}

            def sc_mm(rc, hp):
                lo_t = max(rc - 1, 0)
                hi_t = min(rc + 1, NT - 1)
                b0 = lo_t - (rc - 1)           # 0 or 1
                nb = hi_t - lo_t + 1           # 2 or 3
                csl = slice(b0 * 128, (b0 + nb) * 128)
                ksl = slice(rc * 128, (rc + 1) * 128)
                qsl = slice(lo_t * 128, (hi_t + 1) * 128)
                ps_s = pssc.tile([128, 2, 512], F32, tag="sc")
                for a in range(2):
                    rsl = slice(a * 64, (a + 1) * 64)
                    nc.tensor.matmul(
                        ps_s[:, a, csl],
                        lhsT=kT[rsl, hp, ksl],
                        rhs=qT[rsl, hp, qsl],
                        start=True, stop=True,
                    )
                return ps_s, csl, b0, nb

            def sc_post(rc, hp, ps_s, csl, b0, nb):
                ex = expp.tile([128, 2, 384], BF, tag="expT", name=f"ex{rc}_{hp}")
                nc.scalar.activation(
                    ex[:, :, csl], ps_s[:, :, csl],
                    mybir.ActivationFunctionType.Exp)
                # band masks, multiplicative (both heads in one op);
                # block 1 (diagonal) is fully in-band, no mask needed
                exv = ex[:].rearrange("p a (b q) -> p a b q", q=128)
                if rc > 0 and rc < NT - 1:
                    nc.vector.tensor_mul(
                        exv[:, :, 0:3:2, :], exv[:, :, 0:3:2, :],
                        mskb[:, :, 0:3:2, :])
                elif rc == 0:
                    nc.vector.tensor_mul(
                        exv[:, :, 2:3, :], exv[:, :, 2:3, :],
                        mskb[:, :, 2:3, :])
                else:
                    nc.vector.tensor_mul(
                        exv[:, :, 0:1, :], exv[:, :, 0:1, :],
                        mskb[:, :, 0:1, :])
                expT[(rc, hp)] = ex

            # attention for tile t lags the score pipeline by 2 chunks, so
            # every expT it touches is complete -- its matmuls fill the PE
            # while the current chunk's exp/mask drain on ACT/DVE.
            at_state = {}

            def at_hp(t, hp):
                cs = [c for c in range(3)
                      if not (t == 0 and c == 0) and not (t == NT - 1 and c == 2)]
                if hp == 0:
                    at_state[t] = (
                        work.tile([128, 2, 2, 64], BF, tag="aq", name=f"aq{t}"),
                        pst.tile([128, 2, 128], BF, tag="small", name=f"pst{t}"),
                    )
                aq, ps_t = at_state[t]
                ps_a = psa.tile([128, 2, 65], F32, tag="small2")
                for a in range(2):
                    for i, c in enumerate(cs):
                        ex = expT[(t - 1 + c, hp)]
                        nc.tensor.matmul(
                            ps_a[:, a, :],
                            lhsT=ex[:, a, (2 - c) * 128:(3 - c) * 128],
                            rhs=v4[:, hp * 2 + a, t - 1 + c, :],
                            start=(i == 0), stop=(i == len(cs) - 1),
                        )
                rcp = work.tile([128, 2, 1], F32, tag="rcp")
                nc.vector.reciprocal_approx_fast(rcp[:], ps_a[:, :, 64:65])
                nc.vector.tensor_scalar_mul(
                    aq[:, hp, 0, :], ps_a[:, 0, 0:64], rcp[:, 0, :])
                nc.scalar.mul(aq[:, hp, 1, :], ps_a[:, 1, 0:64], rcp[:, 1, :])

            def at_tr(t, hp):
                aq, ps_t = at_state[t]
                nc.tensor.transpose(ps_t[:, hp, :], aq[:, hp, :, :], ident[:])

            def at_fin(t):
                _, ps_t = at_state.pop(t)
                att = work.tile([128, 2, 128], BF, tag="att")
                nc.vector.tensor_copy(att[:], ps_t[:])
                ps_o = pso.tile([128, 512], F32, tag="out")
                for hp in range(2):
                    nc.tensor.matmul(
                        ps_o[:],
                        lhsT=att[:, hp, :],
                        rhs=ow[:, hp, :],
                        start=(hp == 0), stop=(hp == 1),
                    )
                osb = work.tile([128, 512], BF, tag="osb", bufs=6)
                if t % 2 == 0:
                    nc.vector.tensor_copy(osb[:], ps_o[:])
                else:
                    nc.scalar.copy(osb[:], ps_o[:])
                # tail tiles issue from gpsimd (idle once production ends),
                # dodging the sync queue's serialized descriptor generation
                eng = nc.gpsimd if t >= 12 else nc.sync
                eng.dma_start(out=out_d[t * 128:(t + 1) * 128, :], in_=osb[:])

            def process_rc(rc):
                args0 = sc_mm(rc, 0)
                sc_post(rc, 0, *args0)
                if rc >= 2:
                    at_hp(rc - 2, 0)
                args1 = sc_mm(rc, 1)
                sc_post(rc, 1, *args1)
                if rc >= 2:
                    at_hp(rc - 2, 1)
                    at_tr(rc - 2, 0)
                    at_tr(rc - 2, 1)
                    at_fin(rc - 2)

            # ------- schedule: production block n, then trailing attention ---
            sc_done = -1
            for n in range(4):
                for t in range(4 * n, 4 * n + 4):
                    v_tile(t)
                qk_block(n)
                hi_rc = 4 * n + 2 if n < 3 else NT - 1
                while sc_done < hi_rc:
                    sc_done += 1
                    process_rc(sc_done)
            for t in (NT - 2, NT - 1):
                at_hp(t, 0)
                at_hp(t, 1)
                at_tr(t, 0)
                at_tr(t, 1)
                at_fin(t)

    nc.compile()
    return nc


# ---------------- host prep + run + gather ----------------
def _get_state():
    if "nc" not in _CACHE:
        _CACHE["nc"] = build_nc()
    if "cos" not in _CACHE:
        _CACHE["cos"], _CACHE["sin"] = _rope_tables()
        _CACHE["rotT"] = _rot_matrix_T()
        _CACHE["ident"] = _bf16(np.eye(128, dtype=np.float32))
        _CACHE["maskblk"] = _mask_blocks()
    return _CACHE


def make_in_maps(x, Wqkv_w, out_w):
    st = _get_state()
    halves = [_prep_weights(Wqkv_w, out_w, h) for h in range(2)]
    in_maps = []
    for core in range(N_CORES):
        n, half = core // 2, core % 2
        wqkT, wvT, owT = halves[half]
        xT = _bf16(x[n].T).reshape(4, 128, T)
        in_maps.append({
            "xT": xT, "wqk": wqkT, "wv": wvT, "ow": owT,
            "cosT": st["cos"], "sinT": st["sin"],
            "rotT": st["rotT"], "ident": st["ident"],
            "maskblk": st["maskblk"],
        })
    return in_maps


def gather(results, out_b, dtype):
    outs = []
    for n in range(N_BATCH):
        o = (results[2 * n]["out"].astype(np.float32)
             + results[2 * n + 1]["out"].astype(np.float32) + out_b[None, :])
        outs.append(o)
    return np.stack(outs).astype(dtype, copy=False)


def kernel(x, Wqkv_w, out_w, out_b):
    x = np.asarray(x)
    st = _get_state()
    in_maps = make_in_maps(x, np.asarray(Wqkv_w), np.asarray(out_w))
    res = bass_utils.run_bass_kernel_spmd(
        st["nc"], in_maps, core_ids=list(range(N_CORES)))
    return gather(res.results, np.asarray(out_b), x.dtype)


# revision 22
# speedup vs baseline: 1.0314x; 1.0259x over previous
"""Banded (sliding-window) multi-head attention for Trainium2, 8 NeuronCores.

Problem: x[4, 2048, 512] -> QKV proj -> RoPE -> banded attention
(window [q-127, q+128]) -> out proj.  See reference.py.

Sharding: (batch n, head-half) -> 8 cores.  Each core computes 4 heads of one
batch end-to-end and a partial out-projection (contraction over its 256 hidden
dims); host gather sums the two partials per batch and adds the bias.

On-core pipeline (matmuls bf16, fp32 PSUM accumulation for qk):
  qkT  = Wqk^T.T @ xT        (feat-major, 2 head-packs of 128 partitions)
  RoPE via signed-permutation matmul (rotT = R.T.T @ qkT) + DVE/gpsimd mul/add
  v    = xT.T @ WvT          (token-major, 16 key chunks, ones col for sums)
  scores, key-chunk-stationary: per key chunk rc, scoresT[k, 3 q-tiles]
         = kT_rc.T @ qT[rc-1..rc+1]  (two row-tiled head matmuls, N<=384)
  expT = exp(scoresT) both heads in one activation; band masks applied
         multiplicatively on DVE (bf16 constant tiles)
  attn[q, d] (+ sums via ones column in v) = expT.T @ v   per q tile
  normalize: recip(sums) -> per-partition scale (DVE a=0, ACT a=1)
  PE-transpose attn[q, (2 heads x 64d)] -> attnT[d-pack, q]
  out partial = attnT.T @ owT -> bf16 psum -> DMA out bf16, host upcasts
"""

import numpy as np
import ml_dtypes

import concourse.bass as bass
import concourse.bacc as bacc
import concourse.mybir as mybir
import concourse.tile as tile
from concourse import bass_utils

# ---------------- problem constants (hardcoded per contract) ----------------
N_BATCH = 4
T = 2048
D_MODEL = 512
NHEAD = 8
HEAD_DIM = 64           # also rotary dim
WIN_LO, WIN_HI = 127, 128
N_CORES = 8

NT = T // 128           # 16 query tiles / key chunks of 128
BF = mybir.dt.bfloat16
F8 = mybir.dt.float8e4
F32 = mybir.dt.float32

_CACHE = {}


# ---------------- host-side constant prep ----------------
def _bf16(a):
    return np.ascontiguousarray(a, dtype=np.float32).astype(ml_dtypes.bfloat16)


def _fp8(a):
    return np.ascontiguousarray(a, dtype=np.float32).astype(ml_dtypes.float8_e4m3fn)


def _rope_tables():
    # row p of a 128-partition head-pack corresponds to head dim d = p % 64
    d_idx = np.arange(128) % HEAD_DIM
    f_idx = d_idx % (HEAD_DIM // 2)
    invf = 1.0 / (10000.0 ** (np.arange(0, HEAD_DIM, 2, dtype=np.float32) / HEAD_DIM))
    ang = np.arange(T, dtype=np.float32)[None, :] * invf[f_idx][:, None]  # [128, T]
    return _bf16(np.cos(ang)), _bf16(np.sin(ang))


def _rot_matrix_T():
    # rot_qT = R @ qT with R the rotate_half signed permutation (per 64-dim head)
    R = np.zeros((128, 128), np.float32)
    for p in range(128):
        if p % 64 < 32:
            R[p, p + 32] = -1.0
        else:
            R[p, p - 32] = 1.0
    return _bf16(R.T)


def _mask_blocks():
    # expT_rc block b holds scoresT[key chunk rc, q tile rc-1+b]; k - q offset
    # is 128*(1-b).  Block 0 (keys one chunk right of queries): keep kp <= qp.
    # Block 2 (keys one chunk left): keep kp >= qp+1.  Block 1 (diag): in-band.
    kp = np.arange(128)[:, None]
    qp = np.arange(128)[None, :]
    m0 = (kp <= qp).astype(np.float32)        # right chunk
    m1 = np.ones((128, 128), np.float32)      # diagonal, fully in band
    m2 = (kp >= qp + 1).astype(np.float32)    # left chunk
    m = np.stack([m0, m1, m2])                # [3, 128, 128]
    # duplicate for both heads of a pack: [128, 2(a), 3(b), 128]
    return _bf16(np.broadcast_to(m[None], (2, 3, 128, 128)).transpose(2, 0, 1, 3))


def _prep_weights(Wqkv_w, out_w, half):
    hs = [half * 4 + i for i in range(4)]
    Wq = Wqkv_w[0 * D_MODEL:1 * D_MODEL].reshape(NHEAD, HEAD_DIM, D_MODEL)[hs]
    Wk = Wqkv_w[1 * D_MODEL:2 * D_MODEL].reshape(NHEAD, HEAD_DIM, D_MODEL)[hs]
    Wv = Wqkv_w[2 * D_MODEL:3 * D_MODEL].reshape(NHEAD, HEAD_DIM, D_MODEL)[hs]
    Wq = Wq * np.float32(1.0 / np.sqrt(HEAD_DIM))     # fold attention scale
    # feat order: q(h0,h1), q(h2,h3), k(h0,h1), k(h2,h3)
    wqk = np.concatenate([Wq.reshape(2, 128, D_MODEL), Wk.reshape(2, 128, D_MODEL)], 0)
    wqkT = wqk.reshape(512, D_MODEL).T.reshape(4, 128, 512)       # [xfeat c, 128, feat]
    wvT = Wv.reshape(256, D_MODEL).T.reshape(4, 128, 256)
    owT = out_w[:, half * 256:(half + 1) * 256].T.reshape(2, 128, 512)
    return _bf16(wqkT), _bf16(wvT), _bf16(owT)


# ---------------- bass program ----------------
def build_nc():
    nc = bacc.Bacc("TRN2", debug=False, enable_asserts=False,
                   target_bir_lowering=False)

    xT_d = nc.dram_tensor("xT", [4, 128, T], BF, kind="ExternalInput")
    wqk_d = nc.dram_tensor("wqk", [4, 128, 512], BF, kind="ExternalInput")
    wv_d = nc.dram_tensor("wv", [4, 128, 256], BF, kind="ExternalInput")
    ow_d = nc.dram_tensor("ow", [2, 128, 512], BF, kind="ExternalInput")
    cos_d = nc.dram_tensor("cosT", [128, T], BF, kind="ExternalInput")
    sin_d = nc.dram_tensor("sinT", [128, T], BF, kind="ExternalInput")
    rt_d = nc.dram_tensor("rotT", [128, 128], BF, kind="ExternalInput")
    id_d = nc.dram_tensor("ident", [128, 128], BF, kind="ExternalInput")
    msk_d = nc.dram_tensor("maskblk", [128, 2, 3, 128], BF, kind="ExternalInput")
    out_d = nc.dram_tensor("out", [T, 512], BF, kind="ExternalOutput")

    with tile.TileContext(nc) as tc:
        with (
            tc.tile_pool(name="persist", bufs=1) as pers,
            tc.tile_pool(name="work", bufs=4) as work,
            tc.tile_pool(name="expp", bufs=10) as expp,
            tc.tile_pool(name="psbig", bufs=2, space="PSUM") as psbig,
            tc.tile_pool(name="pssc", bufs=1, space="PSUM") as pssc,
            tc.tile_pool(name="psa", bufs=2, space="PSUM") as psa,
            tc.tile_pool(name="pst", bufs=1, space="PSUM") as pst,
            tc.tile_pool(name="pso", bufs=1, space="PSUM") as pso,
        ):
            # ------- persistent SBUF tensors -------
            xT = pers.tile([128, 4, T], BF)
            wqk = pers.tile([128, 4, 512], BF)
            wv = pers.tile([128, 4, 256], BF)
            ow = pers.tile([128, 2, 512], BF)
            cosb = pers.tile([128, T], BF)
            sinb = pers.tile([128, T], BF)
            rt = pers.tile([128, 128], BF)
            ident = pers.tile([128, 128], BF)
            mskb = pers.tile([128, 2, 3, 128], BF)
            v4 = pers.tile([128, 4, NT, 65], BF)
            qT = pers.tile([128, 2, T], BF)
            kT = pers.tile([128, 2, T], BF)

            # DMA order = first-use order
            # wv descriptor-gen on the scalar queue, in parallel with x0's
            # on sync -- halves the serial DGE latency ahead of the first matmul
            for c in range(4):
                nc.scalar.dma_start(out=wv[:, c, :], in_=wv_d[c])
                nc.sync.dma_start(out=xT[:, c, 0:512], in_=xT_d[c][:, 0:512])
            for c in range(4):
                nc.sync.dma_start(out=wqk[:, c, :], in_=wqk_d[c])
            nc.sync.dma_start(out=rt[:], in_=rt_d[:])
            nc.sync.dma_start(out=cosb[:, 0:512], in_=cos_d[:, 0:512])
            nc.sync.dma_start(out=sinb[:, 0:512], in_=sin_d[:, 0:512])
            for nn in range(1, 4):
                for c in range(4):
                    nc.sync.dma_start(
                        out=xT[:, c, nn * 512:(nn + 1) * 512],
                        in_=xT_d[c][:, nn * 512:(nn + 1) * 512])
            nc.sync.dma_start(out=cosb[:, 512:], in_=cos_d[:, 512:])
            nc.sync.dma_start(out=sinb[:, 512:], in_=sin_d[:, 512:])
            nc.sync.dma_start(out=mskb[:], in_=msk_d[:])
            for c in range(2):
                nc.sync.dma_start(out=ow[:, c, :], in_=ow_d[c])
            nc.sync.dma_start(out=ident[:], in_=id_d[:])

            nc.vector.memset(v4[:, :, :, 64:65], 1.0)   # ones col -> sums

            # ------- production: qkv proj + rope for one 512-token block -----
            def v_tile(t):
                ps_v = psbig.tile([128, 256], F32, tag="big", name="ps_v")
                for c in range(4):
                    nc.tensor.matmul(
                        ps_v[:],
                        lhsT=xT[:, c, t * 128:(t + 1) * 128],
                        rhs=wv[:, c, :],
                        start=(c == 0), stop=(c == 3),
                    )
                nc.vector.tensor_copy(
                    v4[:, :, t, 0:64],
                    ps_v[:].rearrange("p (h d) -> p h d", h=4),
                )

            def qk_block(n):
                # rope for pack m trails the qk matmuls of pack m+1, so the
                # rot matmul never stalls PE on the scalar psum->sbuf cast
                nsl = slice(n * 512, (n + 1) * 512)
                raws = {}

                def rope_tail(m):
                    raw = raws.pop(m)
                    ps_rot = psbig.tile([128, 512], F32, tag="big")
                    nc.tensor.matmul(ps_rot[:], lhsT=rt[:], rhs=raw[:],
                                     start=True, stop=True)
                    t1 = work.tile([128, 512], BF, tag="t1")
                    nc.gpsimd.tensor_mul(t1[:], raw[:], cosb[:, nsl])
                    t2 = work.tile([128, 512], BF, tag="t2")
                    nc.vector.tensor_mul(t2[:], ps_rot[:], sinb[:, nsl])
                    dest = qT[:, m, nsl] if m < 2 else kT[:, m - 2, nsl]
                    if m % 2 == 0:
                        nc.gpsimd.tensor_add(dest, t1[:], t2[:])
                    else:
                        nc.vector.tensor_add(dest, t1[:], t2[:])

                ms = (0, 2, 1, 3)   # hp0's q and k packs first
                for i, m in enumerate(ms):
                    ps_qk = psbig.tile([128, 512], F32, tag="big")
                    for c in range(4):
                        nc.tensor.matmul(
                            ps_qk[:],
                            lhsT=wqk[:, c, m * 128:(m + 1) * 128],
                            rhs=xT[:, c, nsl],
                            start=(c == 0), stop=(c == 3),
                        )
                    raw = work.tile([128, 512], BF, tag="raw")
                    nc.scalar.copy(raw[:], ps_qk[:])
                    raws[m] = raw
                    if i > 0:
                        rope_tail(ms[i - 1])
                rope_tail(ms[3])

            # ------- attention: key-chunk-stationary scores + exp + mask -----
            expT = {}

            def sc_mm(rc, hp):
                lo_t = max(rc - 1, 0)
                hi_t = min(rc + 1, NT - 1)
                b0 = lo_t - (rc - 1)           # 0 or 1
                nb = hi_t - lo_t + 1           # 2 or 3
                csl = slice(b0 * 128, (b0 + nb) * 128)
                ksl = slice(rc * 128, (rc + 1) * 128)
                qsl = slice(lo_t * 128, (hi_t + 1) * 128)
                ps_s = pssc.tile([128, 2, 512], F32, tag="sc")
                for a in range(2):
                    rsl = slice(a * 64, (a + 1) * 64)
                    nc.tensor.matmul(
                        ps_s[:, a, csl],
                        lhsT=kT[rsl, hp, ksl],
                        rhs=qT[rsl, hp, qsl],
                        start=True, stop=True,
                    )
                return ps_s, csl, b0, nb

            def sc_post(rc, hp, ps_s, csl, b0, nb):
                ex = expp.tile([128, 2, 384], BF, tag="expT", name=f"ex{rc}_{hp}")
                nc.scalar.activation(
                    ex[:, :, csl], ps_s[:, :, csl],
                    mybir.ActivationFunctionType.Exp)
                # band masks, multiplicative (both heads in one op);
                # block 1 (diagonal) is fully in-band, no mask needed
                exv = ex[:].rearrange("p a (b q) -> p a b q", q=128)
                if rc > 0 and rc < NT - 1:
                    nc.vector.tensor_mul(
                        exv[:, :, 0:3:2, :], exv[:, :, 0:3:2, :],
                        mskb[:, :, 0:3:2, :])
                elif rc == 0:
                    nc.vector.tensor_mul(
                        exv[:, :, 2:3, :], exv[:, :, 2:3, :],
                        mskb[:, :, 2:3, :])
                else:
                    nc.vector.tensor_mul(
                        exv[:, :, 0:1, :], exv[:, :, 0:1, :],
                        mskb[:, :, 0:1, :])
                expT[(rc, hp)] = ex

            # attention for tile t lags the score pipeline by 2 chunks, so
            # every expT it touches is complete -- its matmuls fill the PE
            # while the current chunk's exp/mask drain on ACT/DVE.
            at_state = {}

            def at_hp(t, hp):
                cs = [c for c in range(3)
                      if not (t == 0 and c == 0) and not (t == NT - 1 and c == 2)]
                if hp == 0:
                    at_state[t] = (
                        work.tile([128, 2, 2, 64], BF, tag="aq", name=f"aq{t}"),
                        pst.tile([128, 2, 128], BF, tag="small", name=f"pst{t}"),
                    )
                aq, ps_t = at_state[t]
                ps_a = psa.tile([128, 2, 65], F32, tag="small2")
                for a in range(2):
                    for i, c in enumerate(cs):
                        ex = expT[(t - 1 + c, hp)]
                        nc.tensor.matmul(
                            ps_a[:, a, :],
                            lhsT=ex[:, a, (2 - c) * 128:(3 - c) * 128],
                            rhs=v4[:, hp * 2 + a, t - 1 + c, :],
                            start=(i == 0), stop=(i == len(cs) - 1),
                        )
                rcp = work.tile([128, 2, 1], F32, tag="rcp")
                nc.vector.reciprocal_approx_fast(rcp[:], ps_a[:, :, 64:65])
                nc.vector.tensor_scalar_mul(
                    aq[:, hp, 0, :], ps_a[:, 0, 0:64], rcp[:, 0, :])
                nc.scalar.mul(aq[:, hp, 1, :], ps_a[:, 1, 0:64], rcp[:, 1, :])

            def at_tr(t, hp):
                aq, ps_t = at_state[t]
                nc.tensor.transpose(ps_t[:, hp, :], aq[:, hp, :, :], ident[:])

            def at_fin(t):
                _, ps_t = at_state.pop(t)
                att = work.tile([128, 2, 128], BF, tag="att")
                nc.vector.tensor_copy(att[:], ps_t[:])
                ps_o = pso.tile([128, 512], F32, tag="out")
                for hp in range(2):
                    nc.tensor.matmul(
                        ps_o[:],
                        lhsT=att[:, hp, :],
                        rhs=ow[:, hp, :],
                        start=(hp == 0), stop=(hp == 1),
                    )
                osb = work.tile([128, 512], BF, tag="osb", bufs=6)
                if t % 2 == 0:
                    nc.vector.tensor_copy(osb[:], ps_o[:])
                else:
                    nc.scalar.copy(osb[:], ps_o[:])
                # tail tiles issue from gpsimd (idle once production ends),
                # dodging the sync queue's serialized descriptor generation
                eng = nc.gpsimd if t >= 12 else nc.sync
                eng.dma_start(out=out_d[t * 128:(t + 1) * 128, :], in_=osb[:])

            def process_rc(rc):
                args0 = sc_mm(rc, 0)
                sc_post(rc, 0, *args0)
                if rc >= 2:
                    at_hp(rc - 2, 0)
                args1 = sc_mm(rc, 1)
                sc_post(rc, 1, *args1)
                if rc >= 2:
                    at_hp(rc - 2, 1)
                    at_tr(rc - 2, 0)
                    at_tr(rc - 2, 1)
                    at_fin(rc - 2)

            # ------- schedule: production block n, then trailing attention ---
            sc_done = -1
            for n in range(4):
                for t in range(4 * n, 4 * n + 4):
                    v_tile(t)
                qk_block(n)
                hi_rc = 4 * n + 2 if n < 3 else NT - 1
                while sc_done < hi_rc:
                    sc_done += 1
                    process_rc(sc_done)
            for t in (NT - 2, NT - 1):
                at_hp(t, 0)
                at_hp(t, 1)
                at_tr(t, 0)
                at_tr(t, 1)
                at_fin(t)

    nc.compile()
    return nc


# ---------------- host prep + run + gather ----------------
def _get_state():
    if "nc" not in _CACHE:
        _CACHE["nc"] = build_nc()
    if "cos" not in _CACHE:
        _CACHE["cos"], _CACHE["sin"] = _rope_tables()
        _CACHE["rotT"] = _rot_matrix_T()
        _CACHE["ident"] = _bf16(np.eye(128, dtype=np.float32))
        _CACHE["maskblk"] = _mask_blocks()
    return _CACHE


def make_in_maps(x, Wqkv_w, out_w):
    st = _get_state()
    halves = [_prep_weights(Wqkv_w, out_w, h) for h in range(2)]
    in_maps = []
    for core in range(N_CORES):
        n, half = core // 2, core % 2
        wqkT, wvT, owT = halves[half]
        xT = _bf16(x[n].T).reshape(4, 128, T)
        in_maps.append({
            "xT": xT, "wqk": wqkT, "wv": wvT, "ow": owT,
            "cosT": st["cos"], "sinT": st["sin"],
            "rotT": st["rotT"], "ident": st["ident"],
            "maskblk": st["maskblk"],
        })
    return in_maps


def gather(results, out_b, dtype):
    outs = []
    for n in range(N_BATCH):
        o = (results[2 * n]["out"].astype(np.float32)
             + results[2 * n + 1]["out"].astype(np.float32) + out_b[None, :])
        outs.append(o)
    return np.stack(outs).astype(dtype, copy=False)


def kernel(x, Wqkv_w, out_w, out_b):
    x = np.asarray(x)
    st = _get_state()
    in_maps = make_in_maps(x, np.asarray(Wqkv_w), np.asarray(out_w))
    res = bass_utils.run_bass_kernel_spmd(
        st["nc"], in_maps, core_ids=list(range(N_CORES)))
    return gather(res.results, np.asarray(out_b), x.dtype)
